# revision 4
# baseline (speedup 1.0000x reference)
"""Multi-head attention Trainium2 kernel (8-core SPMD, no collectives).

Sharding: 8 cores = 4 batches x 2 head-groups (tensor parallel over heads).
Each core receives the full x[b] and the Wq/Wk/Wv/Wo slices for its 8 heads,
computes attention for those heads over ALL 2048 queries, and stores the
PARTIAL output projection (its heads' contribution, fp16, no bias). The host
gather sums the two partials per batch and adds the bias. This removes the
duplicated K/V projection work of a sequence-split sharding: per-core PE work
is 786432 matmul rows (the zero-duplication floor) vs 917504.

Per-core pipeline (matmul inputs bf16, PSUM accumulation fp32):
  A1: V = x @ Wv              -> VP [16 key-blocks, 8 heads, 65] (ones col 64)
  A2: K^T, Q^T per head-pair  -> KT/QT [128 = 2 heads x 64, 4 pb, 2048]
  B:  per (pair, head, qgroup of 1024): S^T[k,q] = KT.T QT; P = exp(0.125 S^T)
      acc[65, 512] halves += P^T V' over 16 key blocks (row 64 = denom);
      normalize by reciprocal -> CT [hv, 4 pb, 2048] bf16
  C:  partial_out[q,:] = CT.T @ Wo_slice, per 128-query block, stored fp16.
      C for query-group g is emitted right after the last pair's g-chains so
      it overlaps the remaining attention.
PSUM (8 banks): sc tag 2x2 banks, acc tag 2x1, fill tag (V/KQ/C psums) 2x1.
Softmax skips max-subtraction: scores ~ N(0,1), exp is safe.
"""

import numpy as np
import ml_dtypes

import concourse.bass as bass
import concourse.bacc as bacc
import concourse.mybir as mybir
import concourse.tile as tile

B, S, D = 4, 2048, 1024
H, DQ, DV = 16, 64, 64
P = 128
HPC = H // 2           # heads per core
NPB = HPC // 2         # head-pair blocks per core (2 heads on 128 partitions)
NDC = D // P           # 8 contraction chunks of D
NKB = S // P           # 16 key blocks
NQB = S // P           # 16 query blocks
NQG = 2                # query groups of 1024 (exp chunk)
QG = S // NQG
NCORES = 8
BF16 = mybir.dt.bfloat16
F16 = mybir.dt.float16
F32 = mybir.dt.float32


def build_nc(reps=1):
    nc = bacc.Bacc("TRN2", target_bir_lowering=False, debug=False,
                   num_devices=NCORES)

    # Host supplies partition-major layouts (see make_in_maps below).
    xT = nc.dram_tensor("xT", [P, NKB, NDC, P], BF16, kind="ExternalInput")
    wk = nc.dram_tensor("wk", [NPB, P, NDC, P], BF16, kind="ExternalInput")
    wq = nc.dram_tensor("wq", [NPB, P, NDC, P], BF16, kind="ExternalInput")
    wv = nc.dram_tensor("wv", [P, NDC, HPC * DV], BF16, kind="ExternalInput")
    wo = nc.dram_tensor("wo", [P, NPB, D], BF16, kind="ExternalInput")
    out = nc.dram_tensor("out", [S, D], F16, kind="ExternalOutput")

    Exp = mybir.ActivationFunctionType.Exp

    with tile.TileContext(nc) as tc:
      for _rep in range(reps):
        with (
            tc.tile_pool(name="persist", bufs=1) as persist,
            tc.tile_pool(name="ptp", bufs=3) as ptp,
            tc.tile_pool(name="nrm", bufs=4) as nrmp,
            tc.tile_pool(name="outp", bufs=2) as outp,
            tc.tile_pool(name="ps", bufs=2,
                         space=bass.MemorySpace.PSUM) as ps,
        ):
            KT = persist.tile([P, NPB, S], BF16, tag="KT")
            QT = persist.tile([P, NPB, S], BF16, tag="QT")
            VP = persist.tile([P, NKB, HPC, DV + 1], BF16, tag="VP")
            CT = persist.tile([P, NPB, S], BF16, tag="CT")
            xt = persist.tile([P, NKB, NDC, P], BF16, tag="xt")
            wvt = persist.tile([P, NDC, HPC * DV], BF16, tag="wv")
            wkt = persist.tile([P, NPB, NDC, P], BF16, tag="wk")
            wqt = persist.tile([P, NPB, NDC, P], BF16, tag="wq")
            wot = persist.tile([P, NPB, D], BF16, tag="wo")

            # ones column of V' (softmax denominator accumulator)
            nc.vector.memset(VP[:, :, :, DV:DV + 1], 1.0)

            # DMA order: wv first (A1 starts earliest), then x key-blocks,
            # then K/Q/O projection weights (needed progressively later).
            nc.sync.dma_start(wvt[:], wv[:])
            for kb in range(4):
                nc.sync.dma_start(xt[:, kb], xT[:, kb])
            nc.sync.dma_start(wkt[:, 0], wk[0])
            nc.sync.dma_start(wqt[:, 0], wq[0])
            for kb in range(4, NKB):
                nc.sync.dma_start(xt[:, kb], xT[:, kb])
            for pb in range(1, NPB):
                nc.sync.dma_start(wkt[:, pb], wk[pb])
                nc.sync.dma_start(wqt[:, pb], wq[pb])
            for pb in range(NPB):
                nc.sync.dma_start(wot[:, pb, :], wo[:, pb, :])

            # ---- A1: V projection, all 8 heads at once ----
            for kb in range(NKB):
                vps = ps.tile([P, HPC * DV], F32, tag="fill", name="vps")
                for dc in range(NDC):
                    nc.tensor.matmul(
                        vps[:],
                        xt[:, kb, dc, :],
                        wvt[:, dc, :],
                        start=(dc == 0), stop=(dc == NDC - 1))
                nc.vector.tensor_copy(
                    VP[:, kb, :, 0:DV],
                    vps[:].rearrange("p (h v) -> p h v", h=HPC))

            # ---- A2 helper: K^T/Q^T projection for one pair block ----
            def project_pb(pb):
                for nb in range(4):
                    kps = ps.tile([P, 512], F32, tag="fill", name="kps")
                    for dc in range(NDC):
                        nc.tensor.matmul(
                            kps[:],
                            wkt[:, pb, dc, :],
                            xt[:, 4 * nb:4 * nb + 4, dc, :],
                            start=(dc == 0), stop=(dc == NDC - 1))
                    nc.vector.tensor_copy(
                        KT[:, pb, nb * 512:(nb + 1) * 512], kps[:])
                for nb in range(4):
                    qps = ps.tile([P, 512], F32, tag="fill", name="qps")
                    for dc in range(NDC):
                        nc.tensor.matmul(
                            qps[:],
                            wqt[:, pb, dc, :],
                            xt[:, 4 * nb:4 * nb + 4, dc, :],
                            start=(dc == 0), stop=(dc == NDC - 1))
                    nc.vector.tensor_copy(
                        QT[:, pb, nb * 512:(nb + 1) * 512], qps[:])

            # ---- C helper: partial out projection for one query block ----
            def out_qb(qb):
                outsb = outp.tile([P, D], F16, tag="out", name="outsb")
                for half in range(2):
                    ops = ps.tile([P, 512], F32, tag="fill", name="ops")
                    for pc in range(NPB):
                        nc.tensor.matmul(
                            ops[:],
                            CT[:, pc, qb * P:(qb + 1) * P],
                            wot[:, pc, half * 512:(half + 1) * 512],
                            start=(pc == 0), stop=(pc == NPB - 1))
                    nc.vector.tensor_copy(
                        outsb[:, half * 512:(half + 1) * 512], ops[:])
                nc.sync.dma_start(out[qb * P:(qb + 1) * P, :], outsb[:])

            # ---- B: attention chains, with next-pb projection as PE filler
            # and phase C interleaved into the last pair block ----
            project_pb(0)
            for pb in range(NPB):
                if pb + 1 < NPB:
                    project_pb(pb + 1)
                for qg in range(NQG):
                    for h in (2 * pb, 2 * pb + 1):
                        hh = (h % 2) * 64
                        accs = [ps.tile([DV + 1, 512], F32, tag="acc",
                                        name=f"acc{g}") for g in range(2)]
                        for kc in range(NKB):
                            sc = ps.tile([P, QG], F32, tag="sc", name="sc")
                            for half in range(2):
                                nc.tensor.matmul(
                                    sc[:, half * 512:(half + 1) * 512],
                                    KT[hh:hh + 64, pb, kc * P:(kc + 1) * P],
                                    QT[hh:hh + 64, pb,
                                       qg * QG + half * 512:
                                       qg * QG + (half + 1) * 512],
                                    start=True, stop=True)
                            pt = ptp.tile([P, QG], BF16, tag="pt", name="pt")
                            nc.scalar.activation(pt[:], sc[:], Exp,
                                                 scale=0.125)
                            for g in range(2):
                                nc.tensor.matmul(
                                    accs[g][:],
                                    VP[:, kc, h, :],
                                    pt[:, g * 512:(g + 1) * 512],
                                    start=(kc == 0), stop=(kc == NKB - 1))
                        for g in range(2):
                            rec = nrmp.tile([1, 512], F32, tag="rec",
                                            name="rec")
                            nc.vector.reciprocal(
                                rec[:], accs[g][DV:DV + 1, :])
                            bc = nrmp.tile([DV, 512], F32, tag="bc",
                                           name="bc")
                            nc.gpsimd.partition_broadcast(bc[:], rec[:])
                            nc.vector.tensor_mul(
                                CT[hh:hh + 64, pb,
                                   qg * QG + g * 512:qg * QG + (g + 1) * 512],
                                accs[g][0:DV, :], bc[:])
                    if pb == NPB - 1:
                        for qb in range(qg * (NQB // NQG),
                                        (qg + 1) * (NQB // NQG)):
                            out_qb(qb)

    nc.compile()
    return nc


def make_in_maps(x, Wq, Wk, Wv, Wo, bo):
    bf = ml_dtypes.bfloat16
    x = np.asarray(x, np.float32)

    def xmajor(xb):  # [S, D] -> [P, NKB, NDC, P]
        return np.ascontiguousarray(
            xb.T.reshape(NDC, P, NKB, P).transpose(1, 2, 0, 3)).astype(bf)

    def wpairs(W, hg):  # [H, D, 64] -> core slice [NPB, P, NDC, P]
        a = (np.asarray(W, np.float32)[hg * HPC:(hg + 1) * HPC]
             .transpose(1, 0, 2).reshape(D, HPC * 64))
        return np.ascontiguousarray(
            a.reshape(NDC, P, NPB, P).transpose(2, 1, 0, 3)).astype(bf)

    def pm(a):  # [D, N] -> partition-major [P, NDC, N]
        return np.ascontiguousarray(
            a.reshape(NDC, P, a.shape[1]).transpose(1, 0, 2)).astype(bf)

    xT_b = [xmajor(x[b]) for b in range(B)]
    Wv_f = np.asarray(Wv, np.float32)
    Wo_f = np.asarray(Wo, np.float32)

    in_maps = []
    for c in range(NCORES):
        b, hg = c // 2, c % 2
        wv_h = pm(Wv_f[hg * HPC:(hg + 1) * HPC]
                  .transpose(1, 0, 2).reshape(D, HPC * DV))
        wo_h = np.ascontiguousarray(
            Wo_f[hg * HPC * DV:(hg + 1) * HPC * DV]
            .reshape(NPB, P, D).transpose(1, 0, 2)).astype(bf)
        in_maps.append({
            "xT": xT_b[b],
            "wk": wpairs(Wk, hg),
            "wq": wpairs(Wq, hg),
            "wv": wv_h,
            "wo": wo_h,
        })
    return in_maps


def kernel(x, Wq, Wk, Wv, Wo, bo):
    from concourse.bass_utils import run_bass_kernel_spmd
    in_maps = make_in_maps(x, Wq, Wk, Wv, Wo, bo)
    nc = build_nc()
    res = run_bass_kernel_spmd(nc, in_maps, list(range(NCORES))).results
    bo_f = np.asarray(bo, np.float32)
    full = np.empty((B, S, D), np.float32)
    for b in range(B):
        full[b] = (np.asarray(res[2 * b]["out"], np.float32)
                   + np.asarray(res[2 * b + 1]["out"], np.float32)
                   + bo_f)
    return full
